# revision 10
# baseline (speedup 1.0000x reference)
"""AscendQwen3Attention (T=2048, HIDDEN=2048, HQ=32, HK=8, D=128) on 8 TRN2 cores.

Tensor-parallel over heads: core i owns q-heads [4i..4i+3] and kv-head i
(GQA rep=4 aligns exactly), w_qkv column-sharded to [2048, 768] per core,
w_o row-sharded to [512, 2048] per core. Each core computes a full [T, HIDDEN]
partial of the output projection; the host sums the 8 partials (unshard).

Device kernel (per core, all matmuls bf16 with fp32 PSUM accumulation):
  1. qkv[t, f] = hiddenT.T @ w_qkv_shard          (tokens on partitions)
  2. rmsnorm + mrope on q/k heads in [t, d] layout (free-axis reductions),
     PE-transpose q/k to [d, t]; v stays [t, d]
  3. attention per head, q-chunks of 512, key-blocks of 128:
       S^T[k, q] = (K^T tile).T @ Q^T       -> psum
       P^T = exp(S^T * D^-0.5)              -> ACT, bf16, causal mask on diag
       num^T[d, q] += V_tile.T(keys) @ P^T  -> psum accumulate
       den[1, q]   += ones.T @ P^T          -> psum accumulate
       attn^T = num^T * (1/den broadcast)
  4. out^T[h, t] partial = w_o_shard.T @ attn^T -> DRAM [2048, 2048] f32

Host: cos/sin mrope tables from positions (f32-exact), all weight/activation
pre-packing into SBUF-image layouts, final sum of partials + transpose.
"""

import os
import sys
import types

sys.path.insert(0, "/opt/trn_rl_repo")

import numpy as np
from ml_dtypes import bfloat16

import concourse.bass as bass
import concourse.bacc as bacc
import concourse.tile as tile
import concourse.mybir as mybir
from concourse.bass_utils import run_bass_kernel_spmd

F32 = mybir.dt.float32
BF16 = mybir.dt.bfloat16
AF = mybir.ActivationFunctionType
ALU = mybir.AluOpType

T = 2048
HIDDEN = 2048
HQ, HK, D = 32, 8, 128
HALF = D // 2
MROPE = (16, 24, 24)
THETA = 1.0e6
EPS = 1e-6
N_CORES = 8
HQL = HQ // N_CORES            # 4 q heads per core
FQKV = (HQL + 2) * D           # 768 qkv features per core
KT = HIDDEN // 128             # 16 contraction tiles
TT = T // 128                  # 16 token tiles
NQC = 4                        # q-chunks of 512
QCW = T // NQC                 # 512
SCALE = float(D) ** -0.5

_CACHED = {}


def _build():
    nc = bacc.Bacc("TRN2", target_bir_lowering=False, debug=False,
                   num_devices=N_CORES)

    NH = HQL + 1  # q heads + k head get rope/norm
    ht_d = nc.dram_tensor("ht", [128, KT * T], BF16, kind="ExternalInput")
    wqkv_d = nc.dram_tensor("wqkv", [128, KT * FQKV], BF16, kind="ExternalInput")
    wo_d = nc.dram_tensor("wo", [128, HQL * HIDDEN], BF16, kind="ExternalInput")
    # cos/sin pre-tiled x5 heads: [128, TT * NH * HALF]
    cos_d = nc.dram_tensor("cos", [128, TT * NH * HALF], F32, kind="ExternalInput")
    sin_d = nc.dram_tensor("sin", [128, TT * NH * HALF], F32, kind="ExternalInput")
    qnw_d = nc.dram_tensor("qnw", [128, 128], F32, kind="ExternalInput")
    knw_d = nc.dram_tensor("knw", [128, 128], F32, kind="ExternalInput")
    mask_d = nc.dram_tensor("mask", [128, 1024], BF16, kind="ExternalInput")
    ident_d = nc.dram_tensor("ident", [128, 128], BF16, kind="ExternalInput")
    out_d = nc.dram_tensor("out", [HIDDEN, T], F32, kind="ExternalOutput")
    out_tiled = out_d.ap().rearrange("(a p) b -> a p b", p=128)

    with tile.TileContext(nc) as tc:
        with (
            tc.tile_pool(name="cst", bufs=1) as cst,
            tc.tile_pool(name="big", bufs=1) as big,
            tc.tile_pool(name="wrk", bufs=2) as wrk,
            tc.tile_pool(name="pmm", bufs=5, space="PSUM") as pmm,
            tc.tile_pool(name="pacc", bufs=2, space="PSUM") as pacc,
            tc.tile_pool(name="pden", bufs=1, space="PSUM") as pden,
        ):
            # ---- persistent SBUF images -------------------------------------
            ht_sb = big.tile([128, KT * T], BF16, tag="ht")
            wqkv_sb = big.tile([128, KT * FQKV], BF16, tag="wqkv")
            wo_sb = big.tile([128, HQL * HIDDEN], BF16, tag="wo")
            CW = NH * HALF  # 320 cos cols per t-tile
            cos_sb = cst.tile([128, TT * CW], F32, tag="cos")
            sin_sb = cst.tile([128, TT * CW], F32, tag="sin")
            qnw_sb = cst.tile([128, 128], F32, tag="qnw")
            knw_sb = cst.tile([128, 128], F32, tag="knw")
            mask_sb = cst.tile([128, 1024], BF16, tag="mask")
            ident_sb = cst.tile([128, 128], BF16, tag="ident")
            ones_sb = cst.tile([128, 1], BF16, tag="ones")
            eps_sb = cst.tile([128, 1], F32, tag="eps")
            kt_sb = big.tile([128, T], BF16, tag="ktr")      # K^T [d, t]
            v_sb = big.tile([128, T], BF16, tag="vsb")       # V   [t, d] tiled

            for kt in range(KT):
                nc.sync.dma_start(ht_sb[:, kt * T:(kt + 1) * T],
                                  ht_d.ap()[:, kt * T:(kt + 1) * T])
                nc.sync.dma_start(wqkv_sb[:, kt * FQKV:(kt + 1) * FQKV],
                                  wqkv_d.ap()[:, kt * FQKV:(kt + 1) * FQKV])
            nc.sync.dma_start(wo_sb[:], wo_d.ap())
            nc.sync.dma_start(cos_sb[:], cos_d.ap())
            nc.sync.dma_start(sin_sb[:], sin_d.ap())
            nc.sync.dma_start(qnw_sb[:], qnw_d.ap())
            nc.sync.dma_start(knw_sb[:], knw_d.ap())
            nc.sync.dma_start(mask_sb[:], mask_d.ap())
            nc.sync.dma_start(ident_sb[:], ident_d.ap())
            nc.vector.memset(ones_sb[:], 1.0)
            nc.vector.memset(eps_sb[:], EPS)

            for qc in range(NQC):
                qt_sb = [wrk.tile([128, QCW], BF16, tag=f"qt{h}", name=f"qt{h}_{qc}")
                         for h in range(HQL)]

                # ---- QKV projection + norm/rope for this chunk's 4 t-tiles --
                for tt_i in range(4 * qc, 4 * qc + 4):
                    psA = pmm.tile([128, 512], F32, tag="mm", name=f"psA_{tt_i}")
                    psB = pmm.tile([128, 512], F32, tag="mm", name=f"psB_{tt_i}")
                    for kt in range(KT):
                        lhsT = ht_sb[:, kt * T + tt_i * 128: kt * T + tt_i * 128 + 128]
                        nc.tensor.matmul(psA[:], lhsT,
                                         wqkv_sb[:, kt * FQKV: kt * FQKV + 512],
                                         start=(kt == 0), stop=(kt == KT - 1))
                        nc.tensor.matmul(psB[:, 0:256], lhsT,
                                         wqkv_sb[:, kt * FQKV + 512: kt * FQKV + 768],
                                         start=(kt == 0), stop=(kt == KT - 1))

                    # rmsnorm: per-head sum(x^2) on ACT, istd on DVE,
                    # normalize into a 5-head-wide xn, then batched rope.
                    def head_ps(h):
                        return psA[:, h * 128:(h + 1) * 128] if h < HQL \
                            else psB[:, 0:128]
                    ssq = []
                    for h in range(NH):
                        sq = wrk.tile([128, 128], F32, tag="sq", bufs=3,
                                      name=f"sq_{tt_i}_{h}")
                        sa = wrk.tile([128, 1], F32, tag="ssq", bufs=6,
                                      name=f"ssq_{tt_i}_{h}")
                        nc.scalar.activation(sq[:], head_ps(h), AF.Square,
                                             accum_out=sa[:])
                        ssq.append(sa)
                    xn_all = wrk.tile([128, NH * 128], F32, tag="xna",
                                      name=f"xna_{tt_i}")
                    for h in range(NH):
                        std = wrk.tile([128, 1], F32, tag="std", bufs=6,
                                       name=f"std_{tt_i}_{h}")
                        nc.scalar.activation(std[:], ssq[h][:], AF.Sqrt,
                                             scale=1.0 / D, bias=eps_sb[:])
                        istd = wrk.tile([128, 1], F32, tag="istd", bufs=6,
                                        name=f"istd_{tt_i}_{h}")
                        nc.vector.reciprocal(istd[:], std[:])
                        nc.vector.scalar_tensor_tensor(
                            xn_all[:, h * 128:(h + 1) * 128], head_ps(h),
                            istd[:], qnw_sb[:] if h < HQL else knw_sb[:],
                            op0=ALU.mult, op1=ALU.mult)

                    # batched rope over the 5 heads (strided 3D APs)
                    rot_all = wrk.tile([128, NH * 128], BF16, tag="rota",
                                       name=f"rota_{tt_i}")
                    xr = xn_all[:].rearrange("p (h d) -> p h d", h=NH)
                    rr = rot_all[:].rearrange("p (h d) -> p h d", h=NH)
                    c5 = cos_sb[:, tt_i * CW:(tt_i + 1) * CW].rearrange(
                        "p (h d) -> p h d", h=NH)
                    s5 = sin_sb[:, tt_i * CW:(tt_i + 1) * CW].rearrange(
                        "p (h d) -> p h d", h=NH)
                    x1 = xr[:, :, 0:HALF]
                    x2 = xr[:, :, HALF:D]
                    ta = wrk.tile([128, NH * HALF], F32, tag="ta",
                                  name=f"ta_{tt_i}")
                    tb = wrk.tile([128, NH * HALF], F32, tag="tb",
                                  name=f"tb_{tt_i}")
                    tar = ta[:].rearrange("p (h d) -> p h d", h=NH)
                    tbr = tb[:].rearrange("p (h d) -> p h d", h=NH)
                    nc.vector.tensor_mul(tar, x1, c5)
                    nc.vector.tensor_mul(tbr, x2, s5)
                    nc.vector.tensor_sub(rr[:, :, 0:HALF], tar, tbr)
                    tc2 = wrk.tile([128, NH * HALF], F32, tag="ta",
                                   name=f"tc_{tt_i}")
                    td = wrk.tile([128, NH * HALF], F32, tag="tb",
                                  name=f"td_{tt_i}")
                    tcr = tc2[:].rearrange("p (h d) -> p h d", h=NH)
                    tdr = td[:].rearrange("p (h d) -> p h d", h=NH)
                    nc.vector.tensor_mul(tcr, x2, c5)
                    nc.vector.tensor_mul(tdr, x1, s5)
                    nc.vector.tensor_add(rr[:, :, HALF:D], tcr, tdr)

                    # transpose each roped head to [d, t]; V straight copy
                    for h in range(NH):
                        tp = pmm.tile([128, 128], BF16, tag="mm",
                                      name=f"tp_{tt_i}_{h}")
                        nc.tensor.transpose(
                            tp[:], rot_all[:, h * 128:(h + 1) * 128],
                            ident_sb[:])
                        if h < HQL:
                            nc.vector.tensor_copy(
                                qt_sb[h][:, (tt_i - 4 * qc) * 128:
                                          (tt_i - 4 * qc) * 128 + 128], tp[:])
                        else:
                            nc.vector.tensor_copy(
                                kt_sb[:, tt_i * 128: tt_i * 128 + 128], tp[:])
                    nc.vector.tensor_copy(
                        v_sb[:, tt_i * 128:(tt_i + 1) * 128], psB[:, 128:256])

                # ---- attention for this q-chunk -----------------------------
                nkb = 4 * qc + 4
                attn_t = []
                for h in range(HQL):
                    o_ps = pacc.tile([128, QCW], F32, tag="o", name=f"o_{qc}_{h}")
                    den_ps = pden.tile([1, QCW], F32, tag="den", name=f"den_{qc}_{h}")
                    for kb in range(nkb):
                        s_ps = pmm.tile([128, QCW], F32, tag="mm",
                                        name=f"s_{qc}_{h}_{kb}")
                        nc.tensor.matmul(s_ps[:],
                                         kt_sb[:, kb * 128:(kb + 1) * 128],
                                         qt_sb[h][:], start=True, stop=True)
                        pt = wrk.tile([128, QCW], BF16, tag="pt", bufs=4,
                                      name=f"pt_{qc}_{h}_{kb}")
                        nc.scalar.activation(pt[:], s_ps[:], AF.Exp, scale=SCALE)
                        r = kb - 4 * qc
                        if r >= 0:
                            nc.vector.tensor_mul(
                                pt[:], pt[:],
                                mask_sb[:, 512 - 128 * r: 1024 - 128 * r])
                        nc.tensor.matmul(o_ps[:],
                                         v_sb[:, kb * 128:(kb + 1) * 128],
                                         pt[:], start=(kb == 0),
                                         stop=(kb == nkb - 1))
                        nc.tensor.matmul(den_ps[0:1, :], ones_sb[:, 0:1],
                                         pt[:], start=(kb == 0),
                                         stop=(kb == nkb - 1))
                    den_r = wrk.tile([1, QCW], F32, tag="denr")
                    den_s = wrk.tile([1, QCW], F32, tag="dens")
                    nc.vector.reciprocal_approx_accurate(
                        den_r[0:1, :], den_ps[0:1, :], den_s[0:1, :])
                    den_b = wrk.tile([128, QCW], F32, tag="denb")
                    nc.gpsimd.partition_broadcast(den_b[:], den_r[0:1, :])
                    at = wrk.tile([128, QCW], BF16, tag=f"at{h}",
                                  name=f"at_{qc}_{h}")
                    nc.vector.tensor_mul(at[:], o_ps[:], den_b[:])
                    attn_t.append(at)

                # ---- output projection for this t-chunk ---------------------
                for ho in range(TT):
                    op_ps = pmm.tile([128, QCW], F32, tag="mm",
                                     name=f"op_{qc}_{ho}")
                    for f in range(HQL):
                        nc.tensor.matmul(
                            op_ps[:],
                            wo_sb[:, f * HIDDEN + ho * 128: f * HIDDEN + ho * 128 + 128],
                            attn_t[f][:], start=(f == 0), stop=(f == HQL - 1))
                    o_sb = wrk.tile([128, QCW], F32, tag="osb", bufs=2,
                                    name=f"osb_{qc}_{ho}")
                    if ho % 2 == 0:
                        nc.vector.tensor_copy(o_sb[:], op_ps[:])
                    else:
                        nc.scalar.copy(o_sb[:], op_ps[:])
                    nc.sync.dma_start(
                        out_tiled[ho][:, qc * QCW:(qc + 1) * QCW], o_sb[:])

    nc.compile()
    return nc


def _pack_rows(a):
    """[N*128, M] -> [128, N*M] SBUF image (partition = row % ... row-major tiles)."""
    n = a.shape[0] // 128
    return np.ascontiguousarray(
        a.reshape(n, 128, a.shape[1]).transpose(1, 0, 2).reshape(128, -1))


def _cos_sin(positions):
    j = np.arange(HALF, dtype=np.float32)
    inv_freq = (np.float32(THETA) ** (-j / np.float32(HALF))).astype(np.float32)
    pos = positions.astype(np.float32)
    freqs3 = pos[:, :, None] * inv_freq[None, None, :]      # [3, T, HALF] f32
    sel = np.zeros(HALF, dtype=np.int64)
    sel[MROPE[0]:MROPE[0] + MROPE[1]] = 1
    sel[MROPE[0] + MROPE[1]:] = 2
    freqs = freqs3[sel, :, np.arange(HALF)].T               # [T, HALF]
    freqs = np.ascontiguousarray(freqs.astype(np.float32))
    return np.cos(freqs).astype(np.float32), np.sin(freqs).astype(np.float32)


def _prep_inputs(hidden_states, positions, w_qkv, w_o, q_norm_w, k_norm_w):
    ht = _pack_rows(np.ascontiguousarray(hidden_states.T).astype(bfloat16))
    cos, sin = _cos_sin(positions)
    nh = HQ // N_CORES + 1
    cos_p = _pack_rows(np.tile(cos, (1, nh)))   # [T, nh*HALF]
    sin_p = _pack_rows(np.tile(sin, (1, nh)))
    qnw = np.tile(np.asarray(q_norm_w, np.float32)[None, :], (128, 1))
    knw = np.tile(np.asarray(k_norm_w, np.float32)[None, :], (128, 1))
    mask = (np.arange(1024)[None, :] >= (np.arange(128)[:, None] + 512)
            ).astype(bfloat16)
    ident = np.eye(128, dtype=bfloat16)

    in_maps = []
    for i in range(N_CORES):
        q0 = 4 * i * D
        wq = w_qkv[:, q0: q0 + HQL * D]
        wk = w_qkv[:, HQ * D + i * D: HQ * D + (i + 1) * D]
        wv = w_qkv[:, (HQ + HK) * D + i * D: (HQ + HK) * D + (i + 1) * D]
        wqkv_i = np.concatenate([wq, wk, wv], axis=1).astype(bfloat16)
        wo_i = w_o[4 * i * D: 4 * (i + 1) * D, :].astype(bfloat16)
        in_maps.append({
            "ht": ht,
            "wqkv": _pack_rows(wqkv_i),
            "wo": _pack_rows(wo_i),
            "cos": cos_p,
            "sin": sin_p,
            "qnw": qnw,
            "knw": knw,
            "mask": mask,
            "ident": ident,
        })
    return in_maps


LAST_RESULTS = None


def kernel(**inputs):
    global LAST_RESULTS
    if "nc" not in _CACHED:
        _CACHED["nc"] = _build()
    nc = _CACHED["nc"]
    in_maps = _prep_inputs(**{k: np.asarray(v) for k, v in inputs.items()})
    trace = bool(os.environ.get("BASS_TRACE"))
    res = run_bass_kernel_spmd(nc, in_maps, core_ids=list(range(N_CORES)),
                               trace=trace)
    LAST_RESULTS = res
    acc = np.zeros((HIDDEN, T), dtype=np.float32)
    for i in range(N_CORES):
        acc += res.results[i]["out"]
    return np.ascontiguousarray(acc.T)


# revision 17
# speedup vs baseline: 1.1597x; 1.1597x over previous
"""AscendQwen3Attention (T=2048, HIDDEN=2048, HQ=32, HK=8, D=128) on 8 TRN2 cores.

Tensor-parallel over heads: core i owns q-heads [4i..4i+3] and kv-head i
(GQA rep=4 aligns exactly), w_qkv column-sharded to [2048, 768] per core,
w_o row-sharded to [512, 2048] per core. Each core computes a full [T, HIDDEN]
partial of the output projection; the host sums the 8 partials (unshard).

Device kernel (per core, all matmuls bf16 with fp32 PSUM accumulation):
  1. qkv[t, f] = hiddenT.T @ w_qkv_shard          (tokens on partitions)
  2. rmsnorm + mrope on q/k heads in [t, d] layout (free-axis reductions),
     PE-transpose q/k to [d, t]; v stays [t, d]
  3. attention per head, q-chunks of 512, key-blocks of 128:
       S^T[k, q] = (K^T tile).T @ Q^T       -> psum
       P^T = exp(S^T * D^-0.5)              -> ACT, bf16, causal mask on diag
       num^T[d, q] += V_tile.T(keys) @ P^T  -> psum accumulate
       den[1, q]   += ones.T @ P^T          -> psum accumulate
       attn^T = num^T * (1/den broadcast)
  4. out^T[h, t] partial = w_o_shard.T @ attn^T -> DRAM [2048, 2048] f32

Host: cos/sin mrope tables from positions (f32-exact), all weight/activation
pre-packing into SBUF-image layouts, final sum of partials + transpose.
"""

import os
import sys
import types

sys.path.insert(0, "/opt/trn_rl_repo")

import numpy as np
from ml_dtypes import bfloat16

import concourse.bass as bass
import concourse.bacc as bacc
import concourse.tile as tile
import concourse.mybir as mybir
from concourse.bass_utils import run_bass_kernel_spmd

F32 = mybir.dt.float32
BF16 = mybir.dt.bfloat16
AF = mybir.ActivationFunctionType
ALU = mybir.AluOpType

T = 2048
HIDDEN = 2048
HQ, HK, D = 32, 8, 128
HALF = D // 2
MROPE = (16, 24, 24)
THETA = 1.0e6
EPS = 1e-6
N_CORES = 8
HQL = HQ // N_CORES            # 4 q heads per core
FQKV = (HQL + 2) * D           # 768 qkv features per core
KT = HIDDEN // 128             # 16 contraction tiles
TT = T // 128                  # 16 token tiles
NQC = 4                        # q-chunks of 512
QCW = T // NQC                 # 512
SCALE = float(D) ** -0.5

_CACHED = {}


def _build():
    nc = bacc.Bacc("TRN2", target_bir_lowering=False, debug=False,
                   num_devices=N_CORES)

    NH = HQL + 1  # q heads + k head get rope/norm
    ht_d = nc.dram_tensor("ht", [128, KT * T], BF16, kind="ExternalInput")
    wqkv_d = nc.dram_tensor("wqkv", [128, KT * FQKV], BF16, kind="ExternalInput")
    wo_d = nc.dram_tensor("wo", [128, HQL * HIDDEN], BF16, kind="ExternalInput")
    # cos/sin pre-tiled x5 heads: [128, TT * NH * HALF]
    cos_d = nc.dram_tensor("cos", [128, TT * NH * HALF], BF16, kind="ExternalInput")
    sin_d = nc.dram_tensor("sin", [128, TT * NH * HALF], BF16, kind="ExternalInput")
    qnw_d = nc.dram_tensor("qnw", [128, 128], F32, kind="ExternalInput")
    knw_d = nc.dram_tensor("knw", [128, 128], F32, kind="ExternalInput")
    mask_d = nc.dram_tensor("mask", [128, 1024], BF16, kind="ExternalInput")
    ident_d = nc.dram_tensor("ident", [128, 128], BF16, kind="ExternalInput")
    out_d = nc.dram_tensor("out", [HIDDEN, T], F32, kind="ExternalOutput")
    out_tiled = out_d.ap().rearrange("(a p) b -> a p b", p=128)

    with tile.TileContext(nc) as tc:
        with (
            tc.tile_pool(name="cst", bufs=1) as cst,
            tc.tile_pool(name="big", bufs=1) as big,
            tc.tile_pool(name="wrk", bufs=2) as wrk,
            tc.tile_pool(name="pqkv", bufs=3, space="PSUM") as pqkv,
            tc.tile_pool(name="pmm", bufs=2, space="PSUM") as pmm,
            tc.tile_pool(name="pacc", bufs=2, space="PSUM") as pacc,
            tc.tile_pool(name="pden", bufs=1, space="PSUM") as pden,
        ):
            # ---- persistent SBUF images -------------------------------------
            ht_sb = big.tile([128, KT * T], BF16, tag="ht")
            wqkv_sb = big.tile([128, KT * FQKV], BF16, tag="wqkv")
            wo_sb = big.tile([128, HQL * HIDDEN], BF16, tag="wo")
            CW = NH * HALF  # 320 cos cols per t-tile
            cos_sb = cst.tile([128, TT * CW], BF16, tag="cos")
            sin_sb = cst.tile([128, TT * CW], BF16, tag="sin")
            qnw_sb = cst.tile([128, 128], F32, tag="qnw")
            knw_sb = cst.tile([128, 128], F32, tag="knw")
            mask_sb = cst.tile([128, 1024], BF16, tag="mask")
            ident_sb = cst.tile([128, 128], BF16, tag="ident")
            ones_sb = cst.tile([128, 1], BF16, tag="ones")
            eps_sb = cst.tile([128, 1], F32, tag="eps")
            kt_sb = big.tile([128, T], BF16, tag="ktr")      # K^T [d, t]
            v_sb = big.tile([128, T], BF16, tag="vsb")       # V   [t, d] tiled

            for kt in range(KT):
                nc.sync.dma_start(ht_sb[:, kt * T:(kt + 1) * T],
                                  ht_d.ap()[:, kt * T:(kt + 1) * T])
                nc.sync.dma_start(wqkv_sb[:, kt * FQKV:(kt + 1) * FQKV],
                                  wqkv_d.ap()[:, kt * FQKV:(kt + 1) * FQKV])
            nc.sync.dma_start(wo_sb[:], wo_d.ap())
            nc.sync.dma_start(cos_sb[:], cos_d.ap())
            nc.sync.dma_start(sin_sb[:], sin_d.ap())
            nc.sync.dma_start(qnw_sb[:], qnw_d.ap())
            nc.sync.dma_start(knw_sb[:], knw_d.ap())
            nc.sync.dma_start(mask_sb[:], mask_d.ap())
            nc.sync.dma_start(ident_sb[:], ident_d.ap())
            nc.vector.memset(ones_sb[:], 1.0)
            nc.vector.memset(eps_sb[:], EPS)

            for qc in range(NQC):
                qt_sb = [wrk.tile([128, QCW], BF16, tag=f"qt{h}", name=f"qt{h}_{qc}")
                         for h in range(HQL)]

                # ---- QKV projection + norm/rope for this chunk's 4 t-tiles --
                for tt_i in range(4 * qc, 4 * qc + 4):
                    psA = pqkv.tile([128, 512], F32, tag="qkv", name=f"psA_{tt_i}")
                    psB = pqkv.tile([128, 512], F32, tag="qkv", name=f"psB_{tt_i}")
                    for kt in range(KT):
                        lhsT = ht_sb[:, kt * T + tt_i * 128: kt * T + tt_i * 128 + 128]
                        nc.tensor.matmul(psA[:], lhsT,
                                         wqkv_sb[:, kt * FQKV: kt * FQKV + 512],
                                         start=(kt == 0), stop=(kt == KT - 1))
                        nc.tensor.matmul(psB[:, 0:256], lhsT,
                                         wqkv_sb[:, kt * FQKV + 512: kt * FQKV + 768],
                                         start=(kt == 0), stop=(kt == KT - 1))
                    # stage psum -> SBUF right away so the banks recycle fast
                    xA = wrk.tile([128, 512], F32, tag="xA", name=f"xA_{tt_i}")
                    xB = wrk.tile([128, 256], F32, tag="xB", name=f"xB_{tt_i}")
                    nc.scalar.copy(xA[:], psA[:])
                    nc.vector.tensor_copy(xB[:], psB[:, 0:256])

                    # rmsnorm: per-head sum(x^2) on ACT, istd on DVE,
                    # normalize into a 5-head-wide xn, then batched rope.
                    def head_ps(h):
                        return xA[:, h * 128:(h + 1) * 128] if h < HQL \
                            else xB[:, 0:128]
                    ssq = []
                    for h in range(NH):
                        sq = wrk.tile([128, 128], F32, tag="sq", bufs=3,
                                      name=f"sq_{tt_i}_{h}")
                        sa = wrk.tile([128, 1], F32, tag="ssq", bufs=6,
                                      name=f"ssq_{tt_i}_{h}")
                        nc.scalar.activation(sq[:], head_ps(h), AF.Square,
                                             accum_out=sa[:])
                        ssq.append(sa)
                    xn_all = wrk.tile([128, NH * 128], F32, tag="xna",
                                      name=f"xna_{tt_i}")
                    for h in range(NH):
                        std = wrk.tile([128, 1], F32, tag="std", bufs=6,
                                       name=f"std_{tt_i}_{h}")
                        nc.scalar.activation(std[:], ssq[h][:], AF.Sqrt,
                                             scale=1.0 / D, bias=eps_sb[:])
                        istd = wrk.tile([128, 1], F32, tag="istd", bufs=6,
                                        name=f"istd_{tt_i}_{h}")
                        nc.vector.reciprocal(istd[:], std[:])
                        nc.vector.scalar_tensor_tensor(
                            xn_all[:, h * 128:(h + 1) * 128], head_ps(h),
                            istd[:], qnw_sb[:] if h < HQL else knw_sb[:],
                            op0=ALU.mult, op1=ALU.mult)

                    # batched rope over the 5 heads (strided 3D APs)
                    rot_all = wrk.tile([128, NH * 128], BF16, tag="rota",
                                       name=f"rota_{tt_i}")
                    xr = xn_all[:].rearrange("p (h d) -> p h d", h=NH)
                    rr = rot_all[:].rearrange("p (h d) -> p h d", h=NH)
                    c5 = cos_sb[:, tt_i * CW:(tt_i + 1) * CW].rearrange(
                        "p (h d) -> p h d", h=NH)
                    s5 = sin_sb[:, tt_i * CW:(tt_i + 1) * CW].rearrange(
                        "p (h d) -> p h d", h=NH)
                    x1 = xr[:, :, 0:HALF]
                    x2 = xr[:, :, HALF:D]
                    ta = wrk.tile([128, NH * HALF], F32, tag="ta",
                                  name=f"ta_{tt_i}")
                    tb = wrk.tile([128, NH * HALF], F32, tag="tb",
                                  name=f"tb_{tt_i}")
                    tar = ta[:].rearrange("p (h d) -> p h d", h=NH)
                    tbr = tb[:].rearrange("p (h d) -> p h d", h=NH)
                    nc.vector.tensor_mul(tar, x1, c5)
                    nc.vector.tensor_mul(tbr, x2, s5)
                    nc.vector.tensor_sub(rr[:, :, 0:HALF], tar, tbr)
                    tc2 = wrk.tile([128, NH * HALF], F32, tag="ta",
                                   name=f"tc_{tt_i}")
                    td = wrk.tile([128, NH * HALF], F32, tag="tb",
                                  name=f"td_{tt_i}")
                    tcr = tc2[:].rearrange("p (h d) -> p h d", h=NH)
                    tdr = td[:].rearrange("p (h d) -> p h d", h=NH)
                    nc.vector.tensor_mul(tcr, x2, c5)
                    nc.vector.tensor_mul(tdr, x1, s5)
                    nc.vector.tensor_add(rr[:, :, HALF:D], tcr, tdr)

                    # transpose each roped head to [d, t]; V straight copy
                    for h in range(NH):
                        tp = pmm.tile([128, 128], BF16, tag="mm",
                                      bufs=2, name=f"tp_{tt_i}_{h}")
                        nc.tensor.transpose(
                            tp[:], rot_all[:, h * 128:(h + 1) * 128],
                            ident_sb[:])
                        if h < HQL:
                            nc.vector.tensor_copy(
                                qt_sb[h][:, (tt_i - 4 * qc) * 128:
                                          (tt_i - 4 * qc) * 128 + 128], tp[:])
                        else:
                            nc.vector.tensor_copy(
                                kt_sb[:, tt_i * 128: tt_i * 128 + 128], tp[:])
                    nc.vector.tensor_copy(
                        v_sb[:, tt_i * 128:(tt_i + 1) * 128], xB[:, 128:256])

                # ---- attention for this q-chunk -----------------------------
                nkb = 4 * qc + 4
                attn_t = []
                for h in range(HQL):
                    o_ps = pacc.tile([128, QCW], F32, tag="o", name=f"o_{qc}_{h}")
                    den_ps = pden.tile([1, QCW], F32, tag="den", name=f"den_{qc}_{h}")
                    for kb in range(nkb):
                        s_ps = pmm.tile([128, QCW], F32, tag="mm",
                                        name=f"s_{qc}_{h}_{kb}")
                        nc.tensor.matmul(s_ps[:],
                                         kt_sb[:, kb * 128:(kb + 1) * 128],
                                         qt_sb[h][:], start=True, stop=True)
                        pt = wrk.tile([128, QCW], BF16, tag="pt", bufs=6,
                                      name=f"pt_{qc}_{h}_{kb}")
                        nc.scalar.activation(pt[:], s_ps[:], AF.Exp, scale=SCALE)
                        r = kb - 4 * qc
                        if r >= 0:
                            nc.vector.tensor_mul(
                                pt[:], pt[:],
                                mask_sb[:, 512 - 128 * r: 1024 - 128 * r])
                        nc.tensor.matmul(o_ps[:],
                                         v_sb[:, kb * 128:(kb + 1) * 128],
                                         pt[:], start=(kb == 0),
                                         stop=(kb == nkb - 1))
                        nc.tensor.matmul(den_ps[0:1, :], ones_sb[:, 0:1],
                                         pt[:], start=(kb == 0),
                                         stop=(kb == nkb - 1))
                    den_r = wrk.tile([1, QCW], F32, tag="denr")
                    den_s = wrk.tile([1, QCW], F32, tag="dens")
                    nc.vector.reciprocal_approx_accurate(
                        den_r[0:1, :], den_ps[0:1, :], den_s[0:1, :])
                    den_b = wrk.tile([128, QCW], F32, tag="denb")
                    nc.gpsimd.partition_broadcast(den_b[:], den_r[0:1, :])
                    at = wrk.tile([128, QCW], BF16, tag=f"at{h}",
                                  name=f"at_{qc}_{h}")
                    nc.vector.tensor_mul(at[:], o_ps[:], den_b[:])
                    attn_t.append(at)

                # ---- output projection for this t-chunk ---------------------
                for ho in range(TT):
                    op_ps = pqkv.tile([128, QCW], F32, tag="qkv",
                                      name=f"op_{qc}_{ho}")
                    for f in range(HQL):
                        nc.tensor.matmul(
                            op_ps[:],
                            wo_sb[:, f * HIDDEN + ho * 128: f * HIDDEN + ho * 128 + 128],
                            attn_t[f][:], start=(f == 0), stop=(f == HQL - 1))
                    o_sb = wrk.tile([128, QCW], F32, tag="osb", bufs=2,
                                    name=f"osb_{qc}_{ho}")
                    if ho % 2 == 0:
                        nc.vector.tensor_copy(o_sb[:], op_ps[:])
                    else:
                        nc.scalar.copy(o_sb[:], op_ps[:])
                    nc.sync.dma_start(
                        out_tiled[ho][:, qc * QCW:(qc + 1) * QCW], o_sb[:])

    nc.compile()
    return nc


def _pack_rows(a):
    """[N*128, M] -> [128, N*M] SBUF image (partition = row % ... row-major tiles)."""
    n = a.shape[0] // 128
    return np.ascontiguousarray(
        a.reshape(n, 128, a.shape[1]).transpose(1, 0, 2).reshape(128, -1))


def _cos_sin(positions):
    j = np.arange(HALF, dtype=np.float32)
    inv_freq = (np.float32(THETA) ** (-j / np.float32(HALF))).astype(np.float32)
    pos = positions.astype(np.float32)
    freqs3 = pos[:, :, None] * inv_freq[None, None, :]      # [3, T, HALF] f32
    sel = np.zeros(HALF, dtype=np.int64)
    sel[MROPE[0]:MROPE[0] + MROPE[1]] = 1
    sel[MROPE[0] + MROPE[1]:] = 2
    freqs = freqs3[sel, :, np.arange(HALF)].T               # [T, HALF]
    freqs = np.ascontiguousarray(freqs.astype(np.float32))
    return np.cos(freqs).astype(np.float32), np.sin(freqs).astype(np.float32)


def _prep_inputs(hidden_states, positions, w_qkv, w_o, q_norm_w, k_norm_w):
    ht = _pack_rows(np.ascontiguousarray(hidden_states.T).astype(bfloat16))
    cos, sin = _cos_sin(positions)
    nh = HQ // N_CORES + 1
    cos_p = _pack_rows(np.tile(cos, (1, nh)).astype(bfloat16))
    sin_p = _pack_rows(np.tile(sin, (1, nh)).astype(bfloat16))
    qnw = np.tile(np.asarray(q_norm_w, np.float32)[None, :], (128, 1))
    knw = np.tile(np.asarray(k_norm_w, np.float32)[None, :], (128, 1))
    mask = (np.arange(1024)[None, :] >= (np.arange(128)[:, None] + 512)
            ).astype(bfloat16)
    ident = np.eye(128, dtype=bfloat16)

    in_maps = []
    for i in range(N_CORES):
        q0 = 4 * i * D
        wq = w_qkv[:, q0: q0 + HQL * D]
        wk = w_qkv[:, HQ * D + i * D: HQ * D + (i + 1) * D]
        wv = w_qkv[:, (HQ + HK) * D + i * D: (HQ + HK) * D + (i + 1) * D]
        wqkv_i = np.concatenate([wq, wk, wv], axis=1).astype(bfloat16)
        wo_i = w_o[4 * i * D: 4 * (i + 1) * D, :].astype(bfloat16)
        in_maps.append({
            "ht": ht,
            "wqkv": _pack_rows(wqkv_i),
            "wo": _pack_rows(wo_i),
            "cos": cos_p,
            "sin": sin_p,
            "qnw": qnw,
            "knw": knw,
            "mask": mask,
            "ident": ident,
        })
    return in_maps


LAST_RESULTS = None


def kernel(**inputs):
    global LAST_RESULTS
    if "nc" not in _CACHED:
        _CACHED["nc"] = _build()
    nc = _CACHED["nc"]
    in_maps = _prep_inputs(**{k: np.asarray(v) for k, v in inputs.items()})
    trace = bool(os.environ.get("BASS_TRACE"))
    res = run_bass_kernel_spmd(nc, in_maps, core_ids=list(range(N_CORES)),
                               trace=trace)
    LAST_RESULTS = res
    acc = np.zeros((HIDDEN, T), dtype=np.float32)
    for i in range(N_CORES):
        acc += res.results[i]["out"]
    return np.ascontiguousarray(acc.T)
